# revision 1
# baseline (speedup 1.0000x reference)
"""Trainium2 Bass kernel for a 2-layer GCN over 2048 independent 25-node
KNN subgraphs (gnn_message_passing).

Strategy:
  - Each 25-node subgraph is independent -> the sparse scatter/gather
    aggregation is a dense per-graph 25x25 matmul. Host packs the
    normalized adjacency (transposed) into block-diagonal 125x125 tiles
    (5 graphs per tile) so the PE array contracts over 125 partitions.
  - Reassociate layer 1: relu(A @ (x @ W0)) == relu((A @ x) @ W0). With
    aggregation first, every matmul stays node-major and no on-chip
    transpose is ever needed (x is fed feature-major from the host).
  - Only the 5 center nodes per tile are needed downstream of the
    layer-2 aggregation, so (A @ h1) is computed for 5 targets only and
    the W1 transform runs once, weight-stationary, over all 260 centers.
  - Data parallel over 8 cores: 256 graphs (52 tiles, last one padded)
    per core; weights replicated.
"""

import os
import sys

import ml_dtypes
import numpy as np

for _p in ("/opt/trn_rl_repo", "/opt/trn_rl_repo/concourse"):
    if _p not in sys.path:
        sys.path.insert(0, _p)

import concourse.bass as bass
import concourse.tile as tile
from concourse import bacc, mybir
from concourse.bass_utils import run_bass_kernel_spmd

NCORES = 8
B = 2048            # graphs
K = 25              # nodes per graph
N = B * K           # 51200
GPC = B // NCORES   # 256 graphs per core
G = 5               # graphs packed per PE tile
P = G * K           # 125 partitions used per tile
NT = (GPC + G - 1) // G   # 52 tiles per core (last tile: 1 real graph)
SLOTS = NT * G      # 260 graph slots per core
NPAD = NT * P       # 6500 padded nodes per core
CP = 8              # padded center count (f32r needs even moving dims)
AW = 128            # at row width (125 block cols + 3 pad for alignment)
F0 = 128            # input features
F1 = 256            # hidden features

_f32 = mybir.dt.float32
_bf16 = mybir.dt.bfloat16

_compiled = {}


def _build_nc(mode):
    """Build + compile the per-core Bass program. mode selects the dtype
    of the SBUF-resident matmul operands:
      f32    - everything float32 (4 cy/col matmuls)
      f32r   - everything float32r (1 cy/col at >=256 moving cols)
      bf16   - everything bfloat16 (1 cy/col any width, fast ldweights)
      hybrid - f32r, but the tiny center-agg matmul (pure overhead) in bf16
    """
    mm_dt = {"f32": _f32, "f32r": mybir.dt.float32r, "bf16": _bf16,
             "hybrid": mybir.dt.float32r}[mode]
    c_dt = _bf16 if mode in ("bf16", "hybrid") else mm_dt
    nc = bacc.Bacc("TRN2", target_bir_lowering=False, debug=False,
                   num_devices=NCORES)

    # Inputs declared with the matmul dtype (float32r is bit-identical to
    # f32; np mapping stays float32) so plain DMAs are not dtype casts.
    xT_d = nc.dram_tensor("xT", [F0, NPAD], mm_dt, kind="ExternalInput")
    # partition-major so the whole tensor DMAs as 125 contiguous 27KB rows
    at_d = nc.dram_tensor("at", [P, NT, AW], mm_dt, kind="ExternalInput")
    w0_d = nc.dram_tensor("w0", [F0, F1], mm_dt, kind="ExternalInput")
    w1_d = nc.dram_tensor("w1", [F1, F1], mm_dt, kind="ExternalInput")
    wl_d = nc.dram_tensor("wl", [128, 2], mm_dt, kind="ExternalInput")
    atc_d = nc.dram_tensor("atc", [P, NT, CP], c_dt, kind="ExternalInput")
    out_d = nc.dram_tensor("out", [1, SLOTS], _f32, kind="ExternalOutput")

    relu = mybir.ActivationFunctionType.Relu

    with tile.TileContext(nc) as tc:
        with (
            tc.tile_pool(name="const", bufs=1) as cpool,
            tc.tile_pool(name="qp", bufs=3) as qp,
            tc.tile_pool(name="h1p", bufs=3) as h1p,
            tc.tile_pool(name="outp", bufs=1) as outp,
            tc.tile_pool(name="ps_q", bufs=2, space=bass.MemorySpace.PSUM) as ps_q,
            tc.tile_pool(name="ps_h1", bufs=2, space=bass.MemorySpace.PSUM) as ps_h1,
            tc.tile_pool(name="ps_p2", bufs=2, space=bass.MemorySpace.PSUM) as ps_p2,
            tc.tile_pool(name="ps_f", bufs=1, space=bass.MemorySpace.PSUM) as ps_f,
        ):
            # ---- resident constants (weights first: tile 0 needs them) ----
            w0 = cpool.tile([F0, F1], mm_dt)
            nc.sync.dma_start(w0[:], w0_d[:])
            w1 = cpool.tile([128, 2 * F1], mm_dt)   # [fic packed] x [fo]
            nc.sync.dma_start(w1[:, 0:F1], w1_d[0:128, :])
            nc.sync.dma_start(w1[:, F1:2 * F1], w1_d[128:256, :])
            wl = cpool.tile([128, 2], mm_dt)
            nc.sync.dma_start(wl[:], wl_d[:])
            xT = cpool.tile([F0, NPAD], mm_dt)
            # layer-2 agg (centers), fT-major, chunk-major so the W1-phase
            # moving operand p2a[:, fi, :] is a contiguous [128, 260]
            p2a = cpool.tile([128, 2, SLOTS], mm_dt)

            at_all = cpool.tile([P, NT, AW], mm_dt)
            atc_all = cpool.tile([P, NT, CP], c_dt)
            nc.sync.dma_start(atc_all[:], atc_d[:])
            nchunk = 8
            cw = NPAD // nchunk
            aw = NT // nchunk   # 6.5 -> use ceil split below
            bounds = [round(NT * c / nchunk) for c in range(nchunk + 1)]
            for c in range(nchunk):
                nc.sync.dma_start(xT[:, c * cw:(c + 1) * cw],
                                  xT_d[:, c * cw:(c + 1) * cw])
                lo, hi = bounds[c], bounds[c + 1]
                nc.sync.dma_start(at_all[:, lo:hi, :], at_d[:, lo:hi, :])

            # ---- per-tile loop ----
            for i in range(NT):
                at_t = at_all[:, i, :]

                # q = x @ W0   (node-major out [125, 256])
                q_ps = ps_q.tile([P, F1], _f32)
                nc.tensor.matmul(q_ps[:], xT[:, i * P:(i + 1) * P], w0[:],
                                 start=True, stop=True)
                q_sb = qp.tile([P, F1], mm_dt)
                nc.vector.tensor_copy(q_sb[:], q_ps[:])

                # h1 = relu(AT.T @ q) = relu(A @ x @ W0)
                h1_ps = ps_h1.tile([P, F1], _f32)
                nc.tensor.matmul(h1_ps[:], at_t[:, 0:P], q_sb[:],
                                 start=True, stop=True)
                h1_sb = h1p.tile([P, F1], c_dt)
                nc.scalar.activation(h1_sb[:], h1_ps[:], relu)

                # p2cT[f, t] = sum_s h1[s, f] * ATc[s, t] (5 centers, padded
                # to 8 cols: f32r rejects small/odd moving free-dims)
                p2_ps = ps_p2.tile([128, 2, CP], _f32)
                for c in range(2):
                    nc.tensor.matmul(p2_ps[:, c, :],
                                     h1_sb[:, c * 128:(c + 1) * 128],
                                     atc_all[:, i, :],
                                     start=True, stop=True)
                nc.vector.tensor_copy(p2a[:, :, i * G:(i + 1) * G],
                                      p2_ps[:, :, 0:G])

            # ---- W1 transform over all centers (weight stationary) ----
            h3_sb = cpool.tile([128, 2, SLOTS], mm_dt)
            for fo in range(2):
                h3_ps = ps_f.tile([128, SLOTS], _f32)
                for fi in range(2):
                    nc.tensor.matmul(h3_ps[:],
                                     w1[:, fi * F1 + fo * 128:fi * F1 + fo * 128 + 128],
                                     p2a[:, fi, :],
                                     start=(fi == 0), stop=(fi == 1))
                nc.scalar.activation(h3_sb[:, fo, :], h3_ps[:], relu)

            # ---- out = relu(h3).T @ Wlin ----
            out_ps = ps_f.tile([1, SLOTS], _f32)
            for fo in range(2):
                nc.tensor.matmul(out_ps[:], wl[:, fo:fo + 1], h3_sb[:, fo, :],
                                 start=(fo == 0), stop=(fo == 1))
            out_sb = outp.tile([1, SLOTS], _f32)
            nc.vector.tensor_copy(out_sb[:], out_ps[:])
            nc.sync.dma_start(out_d[:], out_sb[:])

    nc.compile()
    return nc


def _get_nc(mode):
    if mode not in _compiled:
        _compiled[mode] = _build_nc(mode)
    return _compiled[mode]


def _host_prep(mode, x, edge_weight, W0, W1, Wlin, edge_index):
    mm_np = ml_dtypes.bfloat16 if mode == "bf16" else np.float32
    c_np = ml_dtypes.bfloat16 if mode in ("bf16", "hybrid") else np.float32
    src = edge_index[0].astype(np.int64)
    tgt = edge_index[1].astype(np.int64)
    b = src // K
    sl = src - b * K
    tl = tgt - (tgt // K) * K

    # dense raw adjacency per graph, indexed [b, t, s]
    idx = (b * K + tl) * K + sl
    Araw = np.bincount(idx, weights=edge_weight.astype(np.float64),
                       minlength=B * K * K).astype(np.float32).reshape(B, K, K)
    deg = Araw.sum(axis=2)                      # weighted in-degree [B, K]
    with np.errstate(divide="ignore"):
        dinv = np.where(deg > 0, 1.0 / np.sqrt(deg), 0.0).astype(np.float32)
    An = Araw * dinv[:, :, None] * dinv[:, None, :]   # [b, t, s]
    ATn = np.ascontiguousarray(An.transpose(0, 2, 1))  # [b, s, t]

    # scatter graphs into per-core padded slots
    ATs = np.zeros((NCORES, SLOTS, K, K), np.float32)
    ATs[:, :GPC] = ATn.reshape(NCORES, GPC, K, K)
    ATs = ATs.reshape(NCORES, NT, G, K, K)

    at = np.zeros((NCORES, NT, P, AW), np.float32)
    bd = at[..., :P].reshape(NCORES, NT, G, K, G, K)
    atc = np.zeros((NCORES, NT, P, CP), np.float32)
    cent = atc[..., :G].reshape(NCORES, NT, G, K, G)
    for g in range(G):
        bd[:, :, g, :, g, :] = ATs[:, :, g]          # block-diagonal AT
        cent[:, :, g, :, g] = ATs[:, :, g, :, 0]     # center (t_local=0) col
    # partition-major device layout
    at = np.ascontiguousarray(at.transpose(0, 2, 1, 3).astype(mm_np))
    atc = np.ascontiguousarray(atc.transpose(0, 2, 1, 3).astype(c_np))

    xp = np.zeros((NCORES, NPAD, F0), np.float32)
    xp[:, :GPC * K] = x.reshape(NCORES, GPC * K, F0)
    xT = np.ascontiguousarray(xp.transpose(0, 2, 1).astype(mm_np))

    wl = np.ascontiguousarray(Wlin.reshape(2, 128).T.astype(mm_np))

    in_maps = []
    for c in range(NCORES):
        in_maps.append({
            "xT": xT[c],
            "at": np.ascontiguousarray(at[c]),
            "atc": np.ascontiguousarray(atc[c]),
            "w0": np.ascontiguousarray(W0.astype(mm_np)),
            "w1": np.ascontiguousarray(W1.astype(mm_np)),
            "wl": wl,
        })
    return in_maps


def _run(inputs, mode="f32r", trace=False):
    nc = _get_nc(mode)
    in_maps = _host_prep(mode, **inputs)
    res = run_bass_kernel_spmd(nc, in_maps, core_ids=list(range(NCORES)),
                               trace=trace)
    out = np.empty((B, 1), np.float32)
    for c in range(NCORES):
        out[c * GPC:(c + 1) * GPC, 0] = res.results[c]["out"][0, :GPC]
    return out, res


def kernel(**inputs):
    mode = os.environ.get("GCN_DTYPE", "f32r")
    out, _ = _run(inputs, mode=mode, trace=False)
    return out



# revision 6
# speedup vs baseline: 1.6338x; 1.6338x over previous
"""Trainium2 Bass kernel for a 2-layer GCN over 2048 independent 25-node
KNN subgraphs (gnn_message_passing).

v2 strategy (vs f32r baseline at ~58us):
  - All matmul operands in bf16: 1 cy/col at any width plus FWL (2x
    weight loads; needs 128-col stationary + non-fp32 dtype). PSUM stays
    f32 so accumulation precision is kept; rel-err gate is 2e-2.
  - Reassociate layer 1 as (A @ x) @ W0 computed transposed:
    mT = x_tile.T-stationary x at-moving (128 cols) then
    h1 = mT-stationary x W0-moving (256 cols). This replaces the 256-col
    x@W0 matmul of the baseline with a 128-col aggregation matmul:
    400 moving cols/tile instead of 528, and no operand orientation
    problems anywhere.
  - Everything padded to 128 partitions/columns: uniform APs, FWL on
    every stationary load, fully contiguous per-partition DMAs.
  - PSUM groups of 2 tiles: halves the DVE/Act instruction count (their
    per-instruction fixed cost is 125-260ns). p2 center columns
    accumulate 13 groups per PSUM bank -> 2 copies total instead of 52.
  - Elementwise work split DVE/Act (GpSimd has no PSUM port): DVE does
    the mT casts + p2 copies, relus alternate Act/DVE.
  - DMAs issued in first-use order, each contiguous per partition, so
    the HWDGE FIFO delivers tile 0's data ~1us after the stream starts.
  - PE warmup matmuls on a zeroed tile ramp the PE p-state out of the
    NEFF preamble so real matmuls run at 2.4GHz from the start.
  - Data parallel over 8 cores: 256 graphs (52 tiles of 5 graphs) per
    core; weights replicated.
"""

import sys

import ml_dtypes
import numpy as np

for _p in ("/opt/trn_rl_repo", "/opt/trn_rl_repo/concourse"):
    if _p not in sys.path:
        sys.path.insert(0, _p)

import concourse.bass as bass
import concourse.tile as tile
from concourse import bacc, mybir
from concourse.bass_utils import run_bass_kernel_spmd

NCORES = 8
B = 2048            # graphs
K = 25              # nodes per graph
GPC = B // NCORES   # 256 graphs per core
G = 5               # graphs packed per PE tile
NT = (GPC + G - 1) // G   # 52 tiles per core
SLOTS = NT * G      # 260 graph slots per core
TN = 128            # padded nodes per tile (125 real)
CP = 8              # padded center count per tile (5 real)
F0 = 128            # input features
F1 = 256            # hidden features

GRP = 2             # tiles per PSUM group
NG = NT // GRP      # 26 groups
PHALF = NG // 2     # p2 groups accumulated per PSUM bank
CHUNKS = [6, 6, 8, 16, 16]          # tile counts per streamed DMA chunk
RELU_DVE_MOD = 3    # every Nth group's relu runs on DVE instead of Act
WARMUP = 8          # dummy matmuls to ramp the PE p-state

_f32 = mybir.dt.float32
_bf16 = mybir.dt.bfloat16

_compiled = {}


def _build_nc():
    nc = bacc.Bacc("TRN2", target_bir_lowering=False, debug=False,
                   num_devices=NCORES)

    # node-major x: partitions = node-within-tile, contiguous per partition
    x_d = nc.dram_tensor("x", [TN, NT, F0], _bf16, kind="ExternalInput")
    # at[s, i, t]: block-diagonal normalized adjacency, zero padded
    at_d = nc.dram_tensor("at", [TN, NT, TN], _bf16, kind="ExternalInput")
    atc_d = nc.dram_tensor("atc", [TN, NT, CP], _bf16, kind="ExternalInput")
    w0_d = nc.dram_tensor("w0", [F0, F1], _bf16, kind="ExternalInput")
    w1_d = nc.dram_tensor("w1", [128, 2 * F1], _bf16, kind="ExternalInput")
    wl_d = nc.dram_tensor("wl", [128, 2], _bf16, kind="ExternalInput")
    out_d = nc.dram_tensor("out", [1, SLOTS], _f32, kind="ExternalOutput")

    relu = mybir.ActivationFunctionType.Relu

    with tile.TileContext(nc) as tc:
        with (
            tc.tile_pool(name="const", bufs=1) as cpool,
            tc.tile_pool(name="mtp", bufs=3) as mtp,
            tc.tile_pool(name="h1p", bufs=3) as h1p,
            tc.tile_pool(name="outp", bufs=1) as outp,
            tc.tile_pool(name="ps_mt", bufs=2, space=bass.MemorySpace.PSUM) as ps_mt,
            tc.tile_pool(name="ps_h1", bufs=2, space=bass.MemorySpace.PSUM) as ps_h1,
            tc.tile_pool(name="ps_p2", bufs=2, space=bass.MemorySpace.PSUM) as ps_p2,
            tc.tile_pool(name="ps_f", bufs=1, space=bass.MemorySpace.PSUM) as ps_f,
        ):
            # ---- resident inputs, DMA'd in first-use order ----
            x_all = cpool.tile([TN, NT, F0], _bf16)
            at_all = cpool.tile([TN, NT, TN], _bf16)
            atc_all = cpool.tile([TN, NT, CP], _bf16)
            w0 = cpool.tile([F0, F1], _bf16)
            w1 = cpool.tile([128, 2 * F1], _bf16)
            wl = cpool.tile([128, 2], _bf16)

            bounds = np.cumsum([0] + CHUNKS)
            lo, hi = bounds[0], bounds[1]
            nc.sync.dma_start(x_all[:, lo:hi, :], x_d[:, lo:hi, :])
            nc.sync.dma_start(at_all[:, lo:hi, :], at_d[:, lo:hi, :])
            nc.sync.dma_start(w0[:], w0_d[:])
            nc.sync.dma_start(atc_all[:], atc_d[:])
            for c in range(1, len(CHUNKS)):
                lo, hi = bounds[c], bounds[c + 1]
                nc.sync.dma_start(x_all[:, lo:hi, :], x_d[:, lo:hi, :])
                nc.sync.dma_start(at_all[:, lo:hi, :], at_d[:, lo:hi, :])
                if c == 1:
                    nc.sync.dma_start(w1[:], w1_d[:])
                    nc.sync.dma_start(wl[:], wl_d[:])

            # ---- PE p-state warmup on a zeroed tile (no DMA deps) ----
            warm = cpool.tile([128, 512], _bf16)
            nc.gpsimd.memset(warm[:], 0)
            for _ in range(WARMUP // 2):
                wp = ps_h1.tile([128, GRP, F1], _f32, name="h1_ps")
                for j in range(GRP):
                    nc.tensor.matmul(wp[:, j, :], warm[:, 0:128],
                                     warm[:, 0:F1], start=True, stop=True)

            # p2 accumulator: [f-chunk, tile, center], bf16 for the final
            # weight-stationary W1 transform
            p2a = cpool.tile([128, 2, NT, G], _bf16)

            # ---- per-group loop (GRP tiles per PSUM bank) ----
            p2_ps = None
            for g in range(NG):
                # mT[f, t] = sum_s x[s, f] * at[s, t]  (= (A @ x).T)
                mt_ps = ps_mt.tile([128, GRP, TN], _f32)
                for j in range(GRP):
                    i = g * GRP + j
                    nc.tensor.matmul(mt_ps[:, j, :], x_all[:, i, :],
                                     at_all[:, i, :], start=True, stop=True)
                mt_sb = mtp.tile([128, GRP, TN], _bf16)
                nc.vector.tensor_copy(mt_sb[:], mt_ps[:])

                # h1[t, fo] = sum_f mT[f, t] * W0[f, fo]
                h1_ps = ps_h1.tile([128, GRP, F1], _f32)
                for j in range(GRP):
                    nc.tensor.matmul(h1_ps[:, j, :], mt_sb[:, j, :], w0[:],
                                     start=True, stop=True)
                h1_sb = h1p.tile([128, GRP, F1], _bf16)
                if g % RELU_DVE_MOD == RELU_DVE_MOD - 1:
                    nc.vector.tensor_scalar_max(h1_sb[:], h1_ps[:], 0.0)
                else:
                    nc.scalar.activation(h1_sb[:], h1_ps[:], relu)

                # p2T[f, tc] = sum_s h1[s, f] * ATc[s, tc]; 13 groups
                # accumulate per PSUM bank before one copy out
                if g % PHALF == 0:
                    p2_ps = ps_p2.tile([128, 2, PHALF * GRP, CP], _f32)
                u = (g % PHALF) * GRP
                for j in range(GRP):
                    i = g * GRP + j
                    for c in range(2):
                        nc.tensor.matmul(p2_ps[:, c, u + j, :],
                                         h1_sb[:, j, c * 128:(c + 1) * 128],
                                         atc_all[:, i, :],
                                         start=True, stop=True)
                if g % PHALF == PHALF - 1:
                    h = g // PHALF
                    nc.vector.tensor_copy(
                        p2a[:, :, h * PHALF * GRP:(h + 1) * PHALF * GRP, :],
                        p2_ps[:, :, :, 0:G])

            # ---- W1 transform over all centers (weight stationary) ----
            h3_sb = cpool.tile([128, 2, SLOTS], _bf16)
            for fo in range(2):
                h3_ps = ps_f.tile([128, SLOTS], _f32)
                for fi in range(2):
                    nc.tensor.matmul(h3_ps[:],
                                     w1[:, fi * F1 + fo * 128:fi * F1 + fo * 128 + 128],
                                     p2a[:, fi, :, :],
                                     start=(fi == 0), stop=(fi == 1))
                nc.scalar.activation(h3_sb[:, fo, :], h3_ps[:], relu)

            # ---- out = relu(h3).T @ Wlin ----
            out_ps = ps_f.tile([1, SLOTS], _f32, name="h3_ps")
            for fo in range(2):
                nc.tensor.matmul(out_ps[:], wl[:, fo:fo + 1], h3_sb[:, fo, :],
                                 start=(fo == 0), stop=(fo == 1))
            out_sb = outp.tile([1, SLOTS], _f32)
            nc.vector.tensor_copy(out_sb[:], out_ps[:])
            nc.sync.dma_start(out_d[:], out_sb[:])

    nc.compile()
    return nc


def _get_nc():
    if "nc" not in _compiled:
        _compiled["nc"] = _build_nc()
    return _compiled["nc"]


def _host_prep(x, edge_weight, W0, W1, Wlin, edge_index):
    bf16 = ml_dtypes.bfloat16
    src = edge_index[0].astype(np.int64)
    tgt = edge_index[1].astype(np.int64)
    b = src // K
    sl = src - b * K
    tl = tgt - (tgt // K) * K

    # dense raw adjacency per graph, indexed [b, t, s]
    idx = (b * K + tl) * K + sl
    Araw = np.bincount(idx, weights=edge_weight.astype(np.float64),
                       minlength=B * K * K).astype(np.float32).reshape(B, K, K)
    deg = Araw.sum(axis=2)                      # weighted in-degree [B, K]
    with np.errstate(divide="ignore"):
        dinv = np.where(deg > 0, 1.0 / np.sqrt(deg), 0.0).astype(np.float32)
    An = Araw * dinv[:, :, None] * dinv[:, None, :]   # [b, t, s]
    ATn = np.ascontiguousarray(An.transpose(0, 2, 1))  # [b, s, t]

    # scatter graphs into per-core padded slots
    ATs = np.zeros((NCORES, SLOTS, K, K), np.float32)
    ATs[:, :GPC] = ATn.reshape(NCORES, GPC, K, K)
    ATs = ATs.reshape(NCORES, NT, G, K, K)

    # block-diagonal AT per tile, zero padded to 128x128
    at = np.zeros((NCORES, NT, TN, TN), np.float32)
    bd = at[:, :, :G * K, :G * K].reshape(NCORES, NT, G, K, G, K)
    atc = np.zeros((NCORES, NT, TN, CP), np.float32)
    cent = atc[:, :, :G * K, :G].reshape(NCORES, NT, G, K, G)
    for g in range(G):
        bd[:, :, g, :, g, :] = ATs[:, :, g]          # [s, t] block
        cent[:, :, g, :, g] = ATs[:, :, g, :, 0]     # center (t_local=0) col
    # partition-major (node-within-tile first) device layout
    at = np.ascontiguousarray(at.transpose(0, 2, 1, 3)).astype(bf16)
    atc = np.ascontiguousarray(atc.transpose(0, 2, 1, 3)).astype(bf16)

    # x node-major per tile: [core, s, tile, f]
    xp = np.zeros((NCORES, NT * G * K, F0), np.float32)
    xp[:, :GPC * K] = x.reshape(NCORES, GPC * K, F0)
    xq = np.zeros((NCORES, NT, TN, F0), np.float32)
    xq[:, :, :G * K] = xp.reshape(NCORES, NT, G * K, F0)
    xq = np.ascontiguousarray(xq.transpose(0, 2, 1, 3)).astype(bf16)

    w1 = np.concatenate([W1[0:128, :], W1[128:256, :]], axis=1).astype(bf16)
    wl = np.ascontiguousarray(Wlin.reshape(2, 128).T).astype(bf16)
    w0 = W0.astype(bf16)

    in_maps = []
    for c in range(NCORES):
        in_maps.append({
            "x": xq[c],
            "at": at[c],
            "atc": atc[c],
            "w0": w0,
            "w1": np.ascontiguousarray(w1),
            "wl": wl,
        })
    return in_maps


def _run(inputs, trace=False):
    nc = _get_nc()
    in_maps = _host_prep(**inputs)
    res = run_bass_kernel_spmd(nc, in_maps, core_ids=list(range(NCORES)),
                               trace=trace)
    out = np.empty((B, 1), np.float32)
    for c in range(NCORES):
        out[c * GPC:(c + 1) * GPC, 0] = res.results[c]["out"][0, :GPC]
    return out, res


def kernel(**inputs):
    out, _ = _run(inputs, trace=False)
    return out


# revision 7
# speedup vs baseline: 1.7051x; 1.0436x over previous
"""Trainium2 Bass kernel for a 2-layer GCN over 2048 independent 25-node
KNN subgraphs (gnn_message_passing).

v3 strategy (v2 measured 45.3us traced; baseline f32r 74us traced):
  - bf16 operands everywhere (FWL weight loads, 1 cy/col matmuls),
    f32 PSUM accumulation. rel err ~8e-3 vs 2e-2 gate.
  - Layer 1 reassociated as (A @ x) @ W0 via mT = X.T-stationary x
    AT-moving (128 cols), then h1 = mT-stationary x W0-moving: 400
    moving cols/tile instead of 528 and half the PSUM->SBUF cast bytes.
  - 4-tile stages: one PSUM tile + one cast + one relu instruction per
    4 tiles (Act/DVE fixed cost is ~130-260ns per instruction).
  - Software-pipelined PE stream: stage s issues mm1(s), mm2(s-1),
    p2(s-2) so the in-order PE never sits behind a cast/relu
    round-trip (v2 lost ~570ns/group to exactly that).
  - Cast and relu alternate between DVE and Act per stage to balance
    ~1650ns of elementwise work across both engines.
  - p2 center columns accumulate 16 tiles per PSUM bank -> 4 copies
    total.
  - DMAs split across the two HWDGE rings (SP: adjacency, Act: x) plus
    the GpSimd SWDGE ring (weights + atc): rings run concurrently, so
    the ~0.9us per-DMA ring turnaround overlaps, and first-needed
    chunks land ~1.5us after the preamble.
  - Short PE warmup bridges the gap until the first chunk arrives so
    the HAM activity monitor unthrottles the PE clock (1.2 -> 2.4 GHz)
    early in the loop instead of 14us in (observed in v2).
  - Data parallel over 8 cores: 256 graphs (52 tiles of 5 graphs) per
    core; weights replicated.
"""

import sys

import ml_dtypes
import numpy as np

for _p in ("/opt/trn_rl_repo", "/opt/trn_rl_repo/concourse"):
    if _p not in sys.path:
        sys.path.insert(0, _p)

import concourse.bass as bass
import concourse.tile as tile
from concourse import bacc, mybir
from concourse.bass_utils import run_bass_kernel_spmd

NCORES = 8
B = 2048            # graphs
K = 25              # nodes per graph
GPC = B // NCORES   # 256 graphs per core
G = 5               # graphs packed per PE tile
NT = (GPC + G - 1) // G   # 52 tiles per core
SLOTS = NT * G      # 260 graph slots per core
TN = 128            # padded nodes per tile (125 real)
CP = 8              # padded center count per tile (5 real)
F0 = 128            # input features
F1 = 256            # hidden features

NTS = 4             # tiles per pipeline stage
NS = NT // NTS      # 13 stages
PB = 16             # tiles per p2 PSUM block
CHUNKS = [8, 12, 16, 16]  # tiles per streamed x/at DMA chunk
WARMUP_TILES = 2    # PE warmup PSUM tiles (4 matmuls each)

_f32 = mybir.dt.float32
_bf16 = mybir.dt.bfloat16

_compiled = {}


def _build_nc():
    nc = bacc.Bacc("TRN2", target_bir_lowering=False, debug=False,
                   num_devices=NCORES)

    # node-major x: partitions = node-within-tile, contiguous per partition
    x_d = nc.dram_tensor("x", [TN, NT, F0], _bf16, kind="ExternalInput")
    # at[s, i, t]: block-diagonal normalized adjacency, zero padded
    at_d = nc.dram_tensor("at", [TN, NT, TN], _bf16, kind="ExternalInput")
    atc_d = nc.dram_tensor("atc", [TN, NT, CP], _bf16, kind="ExternalInput")
    w0_d = nc.dram_tensor("w0", [F0, F1], _bf16, kind="ExternalInput")
    w1_d = nc.dram_tensor("w1", [128, 2 * F1], _bf16, kind="ExternalInput")
    wl_d = nc.dram_tensor("wl", [128, 2], _bf16, kind="ExternalInput")
    out_d = nc.dram_tensor("out", [1, SLOTS], _f32, kind="ExternalOutput")

    relu = mybir.ActivationFunctionType.Relu
    copyf = mybir.ActivationFunctionType.Copy

    with tile.TileContext(nc) as tc:
        with (
            tc.tile_pool(name="const", bufs=1) as cpool,
            tc.tile_pool(name="mtp", bufs=3) as mtp,
            tc.tile_pool(name="h1p", bufs=3) as h1p,
            tc.tile_pool(name="outp", bufs=1) as outp,
            tc.tile_pool(name="ps_mt", bufs=2, space=bass.MemorySpace.PSUM) as ps_mt,
            tc.tile_pool(name="ps_h1", bufs=2, space=bass.MemorySpace.PSUM) as ps_h1,
            tc.tile_pool(name="ps_p2", bufs=2, space=bass.MemorySpace.PSUM) as ps_p2,
        ):
            # ---- resident inputs: three concurrent DMA rings, each FIFO,
            # issued in first-use order ----
            x_all = cpool.tile([TN, NT, F0], _bf16)
            at_all = cpool.tile([TN, NT, TN], _bf16)
            atc_all = cpool.tile([TN, NT, CP], _bf16)
            w0 = cpool.tile([F0, F1], _bf16)
            w1 = cpool.tile([128, 2 * F1], _bf16)
            wl = cpool.tile([128, 2], _bf16)

            nc.gpsimd.dma_start(w0[:], w0_d[:])
            nc.gpsimd.dma_start(atc_all[:], atc_d[:])
            nc.gpsimd.dma_start(w1[:], w1_d[:])
            nc.gpsimd.dma_start(wl[:], wl_d[:])
            bounds = np.cumsum([0] + CHUNKS)
            for c in range(len(CHUNKS)):
                lo, hi = bounds[c], bounds[c + 1]
                nc.scalar.dma_start(x_all[:, lo:hi, :], x_d[:, lo:hi, :])
                nc.sync.dma_start(at_all[:, lo:hi, :], at_d[:, lo:hi, :])

            # ---- PE warmup on a zeroed tile (no DMA deps): keeps the PE
            # busy until real data lands so HAM unthrottles early ----
            warm = cpool.tile([128, F1], _bf16)
            nc.vector.memset(warm[:], 0)
            for _ in range(WARMUP_TILES):
                wp = ps_h1.tile([128, NTS, F1], _f32, name="h1_ps")
                for j in range(NTS):
                    nc.tensor.matmul(wp[:, j, :], warm[:, 0:128], warm[:],
                                     start=True, stop=True)

            # p2 accumulator: [f-chunk, tile, center], bf16 for the final
            # weight-stationary W1 transform
            p2a = cpool.tile([128, 2, NT, G], _bf16)

            mt_sbs = {}
            h1_sbs = {}
            p2_ps = None
            # ---- software-pipelined stage loop ----
            for s in range(NS + 2):
                if s < NS:
                    # mT[f, t] = sum_s x[s, f] * at[s, t]  (= (A @ x).T)
                    mt_ps = ps_mt.tile([128, NTS, TN], _f32)
                    for j in range(NTS):
                        i = s * NTS + j
                        nc.tensor.matmul(mt_ps[:, j, :], x_all[:, i, :],
                                         at_all[:, i, :], start=True, stop=True)
                    mt_sb = mtp.tile([128, NTS, TN], _bf16)
                    if s % 2 == 0:
                        nc.vector.tensor_copy(mt_sb[:], mt_ps[:])
                    else:
                        nc.scalar.activation(mt_sb[:], mt_ps[:], copyf)
                    mt_sbs[s] = mt_sb

                if 1 <= s <= NS:
                    # h1[t, fo] = sum_f mT[f, t] * W0[f, fo]
                    g = s - 1
                    mt_sb = mt_sbs.pop(g)
                    h1_ps = ps_h1.tile([128, NTS, F1], _f32, name="h1_ps")
                    for j in range(NTS):
                        nc.tensor.matmul(h1_ps[:, j, :], mt_sb[:, j, :], w0[:],
                                         start=True, stop=True)
                    h1_sb = h1p.tile([128, NTS, F1], _bf16)
                    if s % 2 == 0:
                        nc.scalar.activation(h1_sb[:], h1_ps[:], relu)
                    else:
                        nc.vector.tensor_scalar_max(h1_sb[:], h1_ps[:], 0.0)
                    h1_sbs[g] = h1_sb

                if 2 <= s:
                    # p2T[f, tc] = sum_s h1[s, f] * ATc[s, tc]
                    q = s - 2
                    h1_sb = h1_sbs.pop(q)
                    for j in range(NTS):
                        i = q * NTS + j
                        if i % PB == 0:
                            p2_ps = ps_p2.tile([128, 2, PB, CP], _f32,
                                               name="p2_ps")
                        for c in range(2):
                            nc.tensor.matmul(p2_ps[:, c, i % PB, :],
                                             h1_sb[:, j, c * 128:(c + 1) * 128],
                                             atc_all[:, i, :],
                                             start=True, stop=True)
                        if i % PB == PB - 1 or i == NT - 1:
                            n = i % PB + 1
                            blk = i // PB
                            nc.vector.tensor_copy(
                                p2a[:, :, blk * PB:blk * PB + n, :],
                                p2_ps[:, :, 0:n, 0:G])

            # ---- W1 transform over all centers (weight stationary) ----
            h3_sb = cpool.tile([128, 2, SLOTS], _bf16)
            for fo in range(2):
                h3_ps = ps_p2.tile([128, SLOTS], _f32, name="p2_ps")
                for fi in range(2):
                    nc.tensor.matmul(h3_ps[:],
                                     w1[:, fi * F1 + fo * 128:fi * F1 + fo * 128 + 128],
                                     p2a[:, fi, :, :],
                                     start=(fi == 0), stop=(fi == 1))
                if fo == 0:
                    nc.scalar.activation(h3_sb[:, fo, :], h3_ps[:], relu)
                else:
                    nc.vector.tensor_scalar_max(h3_sb[:, fo, :], h3_ps[:], 0.0)

            # ---- out = relu(h3).T @ Wlin ----
            out_ps = ps_mt.tile([1, SLOTS], _f32, name="mt_ps")
            for fo in range(2):
                nc.tensor.matmul(out_ps[:], wl[:, fo:fo + 1], h3_sb[:, fo, :],
                                 start=(fo == 0), stop=(fo == 1))
            out_sb = outp.tile([1, SLOTS], _f32)
            nc.vector.tensor_copy(out_sb[:], out_ps[:])
            nc.sync.dma_start(out_d[:], out_sb[:])

    nc.compile()
    return nc


def _get_nc():
    if "nc" not in _compiled:
        _compiled["nc"] = _build_nc()
    return _compiled["nc"]


def _host_prep(x, edge_weight, W0, W1, Wlin, edge_index):
    bf16 = ml_dtypes.bfloat16
    src = edge_index[0].astype(np.int64)
    tgt = edge_index[1].astype(np.int64)
    b = src // K
    sl = src - b * K
    tl = tgt - (tgt // K) * K

    # dense raw adjacency per graph, indexed [b, t, s]
    idx = (b * K + tl) * K + sl
    Araw = np.bincount(idx, weights=edge_weight.astype(np.float64),
                       minlength=B * K * K).astype(np.float32).reshape(B, K, K)
    deg = Araw.sum(axis=2)                      # weighted in-degree [B, K]
    with np.errstate(divide="ignore"):
        dinv = np.where(deg > 0, 1.0 / np.sqrt(deg), 0.0).astype(np.float32)
    An = Araw * dinv[:, :, None] * dinv[:, None, :]   # [b, t, s]
    ATn = np.ascontiguousarray(An.transpose(0, 2, 1))  # [b, s, t]

    # scatter graphs into per-core padded slots
    ATs = np.zeros((NCORES, SLOTS, K, K), np.float32)
    ATs[:, :GPC] = ATn.reshape(NCORES, GPC, K, K)
    ATs = ATs.reshape(NCORES, NT, G, K, K)

    # block-diagonal AT per tile, zero padded to 128x128
    at = np.zeros((NCORES, NT, TN, TN), np.float32)
    bd = at[:, :, :G * K, :G * K].reshape(NCORES, NT, G, K, G, K)
    atc = np.zeros((NCORES, NT, TN, CP), np.float32)
    cent = atc[:, :, :G * K, :G].reshape(NCORES, NT, G, K, G)
    for g in range(G):
        bd[:, :, g, :, g, :] = ATs[:, :, g]          # [s, t] block
        cent[:, :, g, :, g] = ATs[:, :, g, :, 0]     # center (t_local=0) col
    # partition-major (node-within-tile first) device layout
    at = np.ascontiguousarray(at.transpose(0, 2, 1, 3)).astype(bf16)
    atc = np.ascontiguousarray(atc.transpose(0, 2, 1, 3)).astype(bf16)

    # x node-major per tile: [core, s, tile, f]
    xp = np.zeros((NCORES, NT * G * K, F0), np.float32)
    xp[:, :GPC * K] = x.reshape(NCORES, GPC * K, F0)
    xq = np.zeros((NCORES, NT, TN, F0), np.float32)
    xq[:, :, :G * K] = xp.reshape(NCORES, NT, G * K, F0)
    xq = np.ascontiguousarray(xq.transpose(0, 2, 1, 3)).astype(bf16)

    w1 = np.concatenate([W1[0:128, :], W1[128:256, :]], axis=1).astype(bf16)
    wl = np.ascontiguousarray(Wlin.reshape(2, 128).T).astype(bf16)
    w0 = W0.astype(bf16)

    in_maps = []
    for c in range(NCORES):
        in_maps.append({
            "x": xq[c],
            "at": at[c],
            "atc": atc[c],
            "w0": w0,
            "w1": np.ascontiguousarray(w1),
            "wl": wl,
        })
    return in_maps


def _run(inputs, trace=False):
    nc = _get_nc()
    in_maps = _host_prep(**inputs)
    res = run_bass_kernel_spmd(nc, in_maps, core_ids=list(range(NCORES)),
                               trace=trace)
    out = np.empty((B, 1), np.float32)
    for c in range(NCORES):
        out[c * GPC:(c + 1) * GPC, 0] = res.results[c]["out"][0, :GPC]
    return out, res


def kernel(**inputs):
    out, _ = _run(inputs, trace=False)
    return out


# revision 10
# speedup vs baseline: 1.8153x; 1.0647x over previous
"""Trainium2 Bass kernel for a 2-layer GCN over 2048 independent 25-node
KNN subgraphs (gnn_message_passing).

v3 strategy (v2 measured 45.3us traced; baseline f32r 74us traced):
  - bf16 operands everywhere (FWL weight loads, 1 cy/col matmuls),
    f32 PSUM accumulation. rel err ~8e-3 vs 2e-2 gate.
  - Layer 1 reassociated as (A @ x) @ W0 via mT = X.T-stationary x
    AT-moving (128 cols), then h1 = mT-stationary x W0-moving: 400
    moving cols/tile instead of 528 and half the PSUM->SBUF cast bytes.
  - 4-tile stages: one PSUM tile + one cast + one relu instruction per
    4 tiles (Act/DVE fixed cost is ~130-260ns per instruction).
  - Software-pipelined PE stream: stage s issues mm1(s), mm2(s-1),
    p2(s-2) so the in-order PE never sits behind a cast/relu
    round-trip (v2 lost ~570ns/group to exactly that).
  - Cast and relu alternate between DVE and Act per stage to balance
    ~1650ns of elementwise work across both engines.
  - p2 center columns accumulate 16 tiles per PSUM bank -> 4 copies
    total.
  - DMAs split across the two HWDGE rings (SP: adjacency, Act: x) plus
    the GpSimd SWDGE ring (weights + atc): rings run concurrently, so
    the ~0.9us per-DMA ring turnaround overlaps, and first-needed
    chunks land ~1.5us after the preamble.
  - Short PE warmup bridges the gap until the first chunk arrives so
    the HAM activity monitor unthrottles the PE clock (1.2 -> 2.4 GHz)
    early in the loop instead of 14us in (observed in v2).
  - Data parallel over 8 cores: 256 graphs (52 tiles of 5 graphs) per
    core; weights replicated.
"""

import sys

import ml_dtypes
import numpy as np

for _p in ("/opt/trn_rl_repo", "/opt/trn_rl_repo/concourse"):
    if _p not in sys.path:
        sys.path.insert(0, _p)

import concourse.bass as bass
import concourse.tile as tile
from concourse import bacc, mybir
from concourse.bass_utils import run_bass_kernel_spmd

NCORES = 8
B = 2048            # graphs
K = 25              # nodes per graph
GPC = B // NCORES   # 256 graphs per core
G = 5               # graphs packed per PE tile
NT = (GPC + G - 1) // G   # 52 tiles per core
SLOTS = NT * G      # 260 graph slots per core
TN = 128            # padded nodes per tile (125 real)
CP = 8              # padded center count per tile (5 real)
F0 = 128            # input features
F1 = 256            # hidden features

NTS = 4             # tiles per pipeline stage
NS = NT // NTS      # 13 stages
PB = 16             # tiles per p2 PSUM block
CHUNKS = [4, 8, 8, 16, 16]  # tiles per streamed x/at DMA chunk
WARMUP_TILES = 2    # PE warmup PSUM tiles (4 matmuls each)
D2 = 2              # software-pipeline depth of the W0 stage
D3 = 4              # software-pipeline depth of the p2 stage

_f32 = mybir.dt.float32
_bf16 = mybir.dt.bfloat16

_compiled = {}


def _build_nc():
    nc = bacc.Bacc("TRN2", target_bir_lowering=False, debug=False,
                   num_devices=NCORES)

    # node-major x: partitions = node-within-tile, contiguous per partition
    x_d = nc.dram_tensor("x", [TN, NT, F0], _bf16, kind="ExternalInput")
    # at[s, i, t]: block-diagonal normalized adjacency, zero padded
    at_d = nc.dram_tensor("at", [TN, NT, TN], _bf16, kind="ExternalInput")
    atc_d = nc.dram_tensor("atc", [TN, NT, CP], _bf16, kind="ExternalInput")
    w0_d = nc.dram_tensor("w0", [F0, F1], _bf16, kind="ExternalInput")
    w1_d = nc.dram_tensor("w1", [128, 2 * F1], _bf16, kind="ExternalInput")
    wl_d = nc.dram_tensor("wl", [128, 2], _bf16, kind="ExternalInput")
    out_d = nc.dram_tensor("out", [1, SLOTS], _f32, kind="ExternalOutput")

    relu = mybir.ActivationFunctionType.Relu
    copyf = mybir.ActivationFunctionType.Copy

    with tile.TileContext(nc) as tc:
        with (
            tc.tile_pool(name="const", bufs=1) as cpool,
            tc.tile_pool(name="mtp", bufs=3) as mtp,
            tc.tile_pool(name="h1p", bufs=3) as h1p,
            tc.tile_pool(name="outp", bufs=1) as outp,
            tc.tile_pool(name="ps_mt", bufs=2, space=bass.MemorySpace.PSUM) as ps_mt,
            tc.tile_pool(name="ps_h1", bufs=2, space=bass.MemorySpace.PSUM) as ps_h1,
            tc.tile_pool(name="ps_p2", bufs=2, space=bass.MemorySpace.PSUM) as ps_p2,
        ):
            # ---- resident inputs: three concurrent DMA rings, each FIFO,
            # issued in first-use order ----
            x_all = cpool.tile([TN, NT, F0], _bf16)
            at_all = cpool.tile([TN, NT, TN], _bf16)
            atc_all = cpool.tile([TN, NT, CP], _bf16)
            w0 = cpool.tile([F0, F1], _bf16)
            w1 = cpool.tile([128, 2 * F1], _bf16)
            wl = cpool.tile([128, 2], _bf16)

            nc.gpsimd.dma_start(w0[:], w0_d[:])
            nc.gpsimd.dma_start(atc_all[:], atc_d[:])
            nc.gpsimd.dma_start(w1[:], w1_d[:])
            nc.gpsimd.dma_start(wl[:], wl_d[:])
            bounds = np.cumsum([0] + CHUNKS)
            # chunk 0 goes on the SP ring (earliest to start) so stage 0
            # can begin right as the warmup drains; x/at then alternate
            # between the two HWDGE rings so both stream concurrently
            lo, hi = bounds[0], bounds[1]
            nc.sync.dma_start(x_all[:, lo:hi, :], x_d[:, lo:hi, :])
            nc.sync.dma_start(at_all[:, lo:hi, :], at_d[:, lo:hi, :])
            for c in range(1, len(CHUNKS)):
                lo, hi = bounds[c], bounds[c + 1]
                nc.scalar.dma_start(x_all[:, lo:hi, :], x_d[:, lo:hi, :])
                nc.sync.dma_start(at_all[:, lo:hi, :], at_d[:, lo:hi, :])

            # ---- PE warmup on a zeroed tile (no DMA deps): keeps the PE
            # busy until real data lands so HAM unthrottles early ----
            warm = cpool.tile([128, F1], _bf16)
            nc.vector.memset(warm[:], 0)
            for _ in range(WARMUP_TILES):
                wp = ps_h1.tile([128, NTS, F1], _f32, name="h1_ps")
                for j in range(NTS):
                    nc.tensor.matmul(wp[:, j, :], warm[:, 0:128], warm[:],
                                     start=True, stop=True)

            # p2 accumulator: [f-chunk, tile, center], bf16 for the final
            # weight-stationary W1 transform
            p2a = cpool.tile([128, 2, NT, G], _bf16)

            h3_sb = cpool.tile([128, 2, SLOTS], _bf16)

            mt_sbs = {}
            h1_sbs = {}
            p2_ps = None
            # ---- software-pipelined stage loop ----
            for s in range(NS + D3 + 1):
                if s < NS:
                    # mT[f, t] = sum_s x[s, f] * at[s, t]  (= (A @ x).T)
                    mt_ps = ps_mt.tile([128, NTS, TN], _f32)
                    for j in range(NTS):
                        i = s * NTS + j
                        nc.tensor.matmul(mt_ps[:, j, :], x_all[:, i, :],
                                         at_all[:, i, :], start=True, stop=True)
                    mt_sb = mtp.tile([128, NTS, TN], _bf16)
                    if s % 2 == 0:
                        nc.vector.tensor_copy(mt_sb[:], mt_ps[:])
                    else:
                        nc.scalar.activation(mt_sb[:], mt_ps[:], copyf)
                    mt_sbs[s] = mt_sb

                if D2 <= s < NS + D2:
                    # h1[t, fo] = sum_f mT[f, t] * W0[f, fo]
                    g = s - D2
                    mt_sb = mt_sbs.pop(g)
                    h1_ps = ps_h1.tile([128, NTS, F1], _f32, name="h1_ps")
                    for j in range(NTS):
                        nc.tensor.matmul(h1_ps[:, j, :], mt_sb[:, j, :], w0[:],
                                         start=True, stop=True)
                    h1_sb = h1p.tile([128, NTS, F1], _bf16)
                    if s % 2 == 0:
                        nc.scalar.activation(h1_sb[:], h1_ps[:], relu)
                    else:
                        nc.vector.tensor_scalar_max(h1_sb[:], h1_ps[:], 0.0)
                    h1_sbs[g] = h1_sb

                if D3 <= s < NS + D3:
                    # p2T[f, tc] = sum_s h1[s, f] * ATc[s, tc]
                    q = s - D3
                    h1_sb = h1_sbs.pop(q)
                    for j in range(NTS):
                        i = q * NTS + j
                        if i % PB == 0:
                            p2_ps = ps_p2.tile([128, 2, PB, CP], _f32,
                                               name="p2_ps")
                        for c in range(2):
                            nc.tensor.matmul(p2_ps[:, c, i % PB, :],
                                             h1_sb[:, j, c * 128:(c + 1) * 128],
                                             atc_all[:, i, :],
                                             start=True, stop=True)
                        if i % PB == PB - 1 or i == NT - 1:
                            # copy this block's centers out, then run its
                            # W1 transform inline so the tail stays short
                            n = i % PB + 1
                            blk = i // PB
                            o = blk * PB * G
                            w = n * G
                            nc.vector.tensor_copy(
                                p2a[:, :, blk * PB:blk * PB + n, :],
                                p2_ps[:, :, 0:n, 0:G])
                            h3_ps = ps_p2.tile([128, 2, PB * G], _f32,
                                               name="p2_ps")
                            for fo in range(2):
                                for fi in range(2):
                                    nc.tensor.matmul(
                                        h3_ps[:, fo, 0:w],
                                        w1[:, fi * F1 + fo * 128:fi * F1 + fo * 128 + 128],
                                        p2a[:, fi, blk * PB:blk * PB + n, :],
                                        start=(fi == 0), stop=(fi == 1))
                            if blk % 2 == 0:
                                nc.scalar.activation(
                                    h3_sb[:, :, o:o + w], h3_ps[:, :, 0:w],
                                    relu)
                            else:
                                nc.vector.tensor_scalar_max(
                                    h3_sb[:, :, o:o + w], h3_ps[:, :, 0:w],
                                    0.0)

            # ---- out = relu(h3).T @ Wlin ----
            out_ps = ps_mt.tile([1, SLOTS], _f32, name="mt_ps")
            for fo in range(2):
                nc.tensor.matmul(out_ps[:], wl[:, fo:fo + 1], h3_sb[:, fo, :],
                                 start=(fo == 0), stop=(fo == 1))
            out_sb = outp.tile([1, SLOTS], _f32)
            nc.vector.tensor_copy(out_sb[:], out_ps[:])
            nc.sync.dma_start(out_d[:], out_sb[:])

    nc.compile()
    return nc


def _get_nc():
    if "nc" not in _compiled:
        _compiled["nc"] = _build_nc()
    return _compiled["nc"]


def _host_prep(x, edge_weight, W0, W1, Wlin, edge_index):
    bf16 = ml_dtypes.bfloat16
    src = edge_index[0].astype(np.int64)
    tgt = edge_index[1].astype(np.int64)
    b = src // K
    sl = src - b * K
    tl = tgt - (tgt // K) * K

    # dense raw adjacency per graph, indexed [b, t, s]
    idx = (b * K + tl) * K + sl
    Araw = np.bincount(idx, weights=edge_weight.astype(np.float64),
                       minlength=B * K * K).astype(np.float32).reshape(B, K, K)
    deg = Araw.sum(axis=2)                      # weighted in-degree [B, K]
    with np.errstate(divide="ignore"):
        dinv = np.where(deg > 0, 1.0 / np.sqrt(deg), 0.0).astype(np.float32)
    An = Araw * dinv[:, :, None] * dinv[:, None, :]   # [b, t, s]
    ATn = np.ascontiguousarray(An.transpose(0, 2, 1))  # [b, s, t]

    # scatter graphs into per-core padded slots
    ATs = np.zeros((NCORES, SLOTS, K, K), np.float32)
    ATs[:, :GPC] = ATn.reshape(NCORES, GPC, K, K)
    ATs = ATs.reshape(NCORES, NT, G, K, K)

    # block-diagonal AT per tile, zero padded to 128x128
    at = np.zeros((NCORES, NT, TN, TN), np.float32)
    bd = at[:, :, :G * K, :G * K].reshape(NCORES, NT, G, K, G, K)
    atc = np.zeros((NCORES, NT, TN, CP), np.float32)
    cent = atc[:, :, :G * K, :G].reshape(NCORES, NT, G, K, G)
    for g in range(G):
        bd[:, :, g, :, g, :] = ATs[:, :, g]          # [s, t] block
        cent[:, :, g, :, g] = ATs[:, :, g, :, 0]     # center (t_local=0) col
    # partition-major (node-within-tile first) device layout
    at = np.ascontiguousarray(at.transpose(0, 2, 1, 3)).astype(bf16)
    atc = np.ascontiguousarray(atc.transpose(0, 2, 1, 3)).astype(bf16)

    # x node-major per tile: [core, s, tile, f]
    xp = np.zeros((NCORES, NT * G * K, F0), np.float32)
    xp[:, :GPC * K] = x.reshape(NCORES, GPC * K, F0)
    xq = np.zeros((NCORES, NT, TN, F0), np.float32)
    xq[:, :, :G * K] = xp.reshape(NCORES, NT, G * K, F0)
    xq = np.ascontiguousarray(xq.transpose(0, 2, 1, 3)).astype(bf16)

    w1 = np.concatenate([W1[0:128, :], W1[128:256, :]], axis=1).astype(bf16)
    wl = np.ascontiguousarray(Wlin.reshape(2, 128).T).astype(bf16)
    w0 = W0.astype(bf16)

    in_maps = []
    for c in range(NCORES):
        in_maps.append({
            "x": xq[c],
            "at": at[c],
            "atc": atc[c],
            "w0": w0,
            "w1": np.ascontiguousarray(w1),
            "wl": wl,
        })
    return in_maps


def _run(inputs, trace=False):
    nc = _get_nc()
    in_maps = _host_prep(**inputs)
    res = run_bass_kernel_spmd(nc, in_maps, core_ids=list(range(NCORES)),
                               trace=trace)
    out = np.empty((B, 1), np.float32)
    for c in range(NCORES):
        out[c * GPC:(c + 1) * GPC, 0] = res.results[c]["out"][0, :GPC]
    return out, res


def kernel(**inputs):
    out, _ = _run(inputs, trace=False)
    return out


# revision 11
# speedup vs baseline: 1.8281x; 1.0071x over previous
"""Trainium2 Bass kernel for a 2-layer GCN over 2048 independent 25-node
KNN subgraphs (gnn_message_passing).

v3 strategy (v2 measured 45.3us traced; baseline f32r 74us traced):
  - bf16 operands everywhere (FWL weight loads, 1 cy/col matmuls),
    f32 PSUM accumulation. rel err ~8e-3 vs 2e-2 gate.
  - Layer 1 reassociated as (A @ x) @ W0 via mT = X.T-stationary x
    AT-moving (128 cols), then h1 = mT-stationary x W0-moving: 400
    moving cols/tile instead of 528 and half the PSUM->SBUF cast bytes.
  - 4-tile stages: one PSUM tile + one cast + one relu instruction per
    4 tiles (Act/DVE fixed cost is ~130-260ns per instruction).
  - Software-pipelined PE stream: stage s issues mm1(s), mm2(s-1),
    p2(s-2) so the in-order PE never sits behind a cast/relu
    round-trip (v2 lost ~570ns/group to exactly that).
  - Cast and relu alternate between DVE and Act per stage to balance
    ~1650ns of elementwise work across both engines.
  - p2 center columns accumulate 16 tiles per PSUM bank -> 4 copies
    total.
  - DMAs split across the two HWDGE rings (SP: adjacency, Act: x) plus
    the GpSimd SWDGE ring (weights + atc): rings run concurrently, so
    the ~0.9us per-DMA ring turnaround overlaps, and first-needed
    chunks land ~1.5us after the preamble.
  - Short PE warmup bridges the gap until the first chunk arrives so
    the HAM activity monitor unthrottles the PE clock (1.2 -> 2.4 GHz)
    early in the loop instead of 14us in (observed in v2).
  - Data parallel over 8 cores: 256 graphs (52 tiles of 5 graphs) per
    core; weights replicated.
"""

import sys

import ml_dtypes
import numpy as np

for _p in ("/opt/trn_rl_repo", "/opt/trn_rl_repo/concourse"):
    if _p not in sys.path:
        sys.path.insert(0, _p)

import concourse.bass as bass
import concourse.tile as tile
from concourse import bacc, mybir
from concourse.bass_utils import run_bass_kernel_spmd

NCORES = 8
B = 2048            # graphs
K = 25              # nodes per graph
GPC = B // NCORES   # 256 graphs per core
G = 5               # graphs packed per PE tile
NT = (GPC + G - 1) // G   # 52 tiles per core
SLOTS = NT * G      # 260 graph slots per core
TN = 128            # padded nodes per tile (125 real)
CP = 8              # padded center count per tile (5 real)
F0 = 128            # input features
F1 = 256            # hidden features

NTS = 4             # tiles per pipeline stage
NS = NT // NTS      # 13 stages
PB = 16             # tiles per p2 PSUM block
CHUNKS = [4, 8, 8, 16, 16]  # tiles per streamed x/at DMA chunk
WARMUP_TILES = 6    # PE warmup PSUM tiles (4 matmuls each)
D2 = 2              # software-pipeline depth of the W0 stage
D3 = 4              # software-pipeline depth of the p2 stage

_f32 = mybir.dt.float32
_bf16 = mybir.dt.bfloat16

_compiled = {}


def _build_nc():
    nc = bacc.Bacc("TRN2", target_bir_lowering=False, debug=False,
                   num_devices=NCORES)

    # node-major x: partitions = node-within-tile, contiguous per partition
    x_d = nc.dram_tensor("x", [TN, NT, F0], _bf16, kind="ExternalInput")
    # at[s, i, t]: block-diagonal normalized adjacency, zero padded
    at_d = nc.dram_tensor("at", [TN, NT, TN], _bf16, kind="ExternalInput")
    atc_d = nc.dram_tensor("atc", [TN, NT, CP], _bf16, kind="ExternalInput")
    w0_d = nc.dram_tensor("w0", [F0, F1], _bf16, kind="ExternalInput")
    w1_d = nc.dram_tensor("w1", [128, 2 * F1], _bf16, kind="ExternalInput")
    wl_d = nc.dram_tensor("wl", [128, 2], _bf16, kind="ExternalInput")
    out_d = nc.dram_tensor("out", [1, SLOTS], _f32, kind="ExternalOutput")

    relu = mybir.ActivationFunctionType.Relu
    copyf = mybir.ActivationFunctionType.Copy

    with tile.TileContext(nc) as tc:
        with (
            tc.tile_pool(name="const", bufs=1) as cpool,
            tc.tile_pool(name="mtp", bufs=3) as mtp,
            tc.tile_pool(name="h1p", bufs=3) as h1p,
            tc.tile_pool(name="outp", bufs=1) as outp,
            tc.tile_pool(name="ps_mt", bufs=2, space=bass.MemorySpace.PSUM) as ps_mt,
            tc.tile_pool(name="ps_h1", bufs=2, space=bass.MemorySpace.PSUM) as ps_h1,
            tc.tile_pool(name="ps_p2", bufs=2, space=bass.MemorySpace.PSUM) as ps_p2,
        ):
            # ---- resident inputs: three concurrent DMA rings, each FIFO,
            # issued in first-use order ----
            x_all = cpool.tile([TN, NT, F0], _bf16)
            at_all = cpool.tile([TN, NT, TN], _bf16)
            atc_all = cpool.tile([TN, NT, CP], _bf16)
            w0 = cpool.tile([F0, F1], _bf16)
            w1 = cpool.tile([128, 2 * F1], _bf16)
            wl = cpool.tile([128, 2], _bf16)

            nc.gpsimd.dma_start(w0[:], w0_d[:])
            nc.gpsimd.dma_start(atc_all[:], atc_d[:])
            nc.gpsimd.dma_start(w1[:], w1_d[:])
            nc.gpsimd.dma_start(wl[:], wl_d[:])
            bounds = np.cumsum([0] + CHUNKS)
            # chunk 0 goes on the SP ring (earliest to start) so stage 0
            # can begin right as the warmup drains; x/at then alternate
            # between the two HWDGE rings so both stream concurrently
            lo, hi = bounds[0], bounds[1]
            nc.sync.dma_start(x_all[:, lo:hi, :], x_d[:, lo:hi, :])
            nc.sync.dma_start(at_all[:, lo:hi, :], at_d[:, lo:hi, :])
            for c in range(1, len(CHUNKS)):
                lo, hi = bounds[c], bounds[c + 1]
                nc.scalar.dma_start(x_all[:, lo:hi, :], x_d[:, lo:hi, :])
                nc.sync.dma_start(at_all[:, lo:hi, :], at_d[:, lo:hi, :])

            # ---- PE warmup on a zeroed tile (no DMA deps): keeps the PE
            # busy until real data lands so HAM unthrottles early ----
            warm = cpool.tile([128, F1], _bf16)
            nc.vector.memset(warm[:], 0)
            for _ in range(WARMUP_TILES):
                wp = ps_h1.tile([128, NTS, F1], _f32, name="h1_ps")
                for j in range(NTS):
                    nc.tensor.matmul(wp[:, j, :], warm[:, 0:128], warm[:],
                                     start=True, stop=True)

            # p2 accumulator: [f-chunk, tile, center], bf16 for the final
            # weight-stationary W1 transform
            p2a = cpool.tile([128, 2, NT, G], _bf16)

            h3_sb = cpool.tile([128, 2, SLOTS], _bf16)

            mt_sbs = {}
            h1_sbs = {}
            p2_ps = None
            # ---- software-pipelined stage loop ----
            for s in range(NS + D3 + 1):
                if s < NS:
                    # mT[f, t] = sum_s x[s, f] * at[s, t]  (= (A @ x).T)
                    mt_ps = ps_mt.tile([128, NTS, TN], _f32)
                    for j in range(NTS):
                        i = s * NTS + j
                        nc.tensor.matmul(mt_ps[:, j, :], x_all[:, i, :],
                                         at_all[:, i, :], start=True, stop=True)
                    mt_sb = mtp.tile([128, NTS, TN], _bf16)
                    if s % 2 == 0:
                        nc.vector.tensor_copy(mt_sb[:], mt_ps[:])
                    else:
                        nc.scalar.activation(mt_sb[:], mt_ps[:], copyf)
                    mt_sbs[s] = mt_sb

                if D2 <= s < NS + D2:
                    # h1[t, fo] = sum_f mT[f, t] * W0[f, fo]
                    g = s - D2
                    mt_sb = mt_sbs.pop(g)
                    h1_ps = ps_h1.tile([128, NTS, F1], _f32, name="h1_ps")
                    for j in range(NTS):
                        nc.tensor.matmul(h1_ps[:, j, :], mt_sb[:, j, :], w0[:],
                                         start=True, stop=True)
                    h1_sb = h1p.tile([128, NTS, F1], _bf16)
                    if s % 2 == 0:
                        nc.scalar.activation(h1_sb[:], h1_ps[:], relu)
                    else:
                        nc.vector.tensor_scalar_max(h1_sb[:], h1_ps[:], 0.0)
                    h1_sbs[g] = h1_sb

                if D3 <= s < NS + D3:
                    # p2T[f, tc] = sum_s h1[s, f] * ATc[s, tc]
                    q = s - D3
                    h1_sb = h1_sbs.pop(q)
                    for j in range(NTS):
                        i = q * NTS + j
                        if i % PB == 0:
                            p2_ps = ps_p2.tile([128, 2, PB, CP], _f32,
                                               name="p2_ps")
                        for c in range(2):
                            nc.tensor.matmul(p2_ps[:, c, i % PB, :],
                                             h1_sb[:, j, c * 128:(c + 1) * 128],
                                             atc_all[:, i, :],
                                             start=True, stop=True)
                        if i % PB == PB - 1 or i == NT - 1:
                            # copy this block's centers out, then run its
                            # W1 transform inline so the tail stays short
                            n = i % PB + 1
                            blk = i // PB
                            o = blk * PB * G
                            w = n * G
                            nc.vector.tensor_copy(
                                p2a[:, :, blk * PB:blk * PB + n, :],
                                p2_ps[:, :, 0:n, 0:G])
                            h3_ps = ps_p2.tile([128, 2, PB * G], _f32,
                                               name="p2_ps")
                            for fo in range(2):
                                for fi in range(2):
                                    nc.tensor.matmul(
                                        h3_ps[:, fo, 0:w],
                                        w1[:, fi * F1 + fo * 128:fi * F1 + fo * 128 + 128],
                                        p2a[:, fi, blk * PB:blk * PB + n, :],
                                        start=(fi == 0), stop=(fi == 1))
                            if blk % 2 == 0:
                                nc.scalar.activation(
                                    h3_sb[:, :, o:o + w], h3_ps[:, :, 0:w],
                                    relu)
                            else:
                                nc.vector.tensor_scalar_max(
                                    h3_sb[:, :, o:o + w], h3_ps[:, :, 0:w],
                                    0.0)

            # ---- out = relu(h3).T @ Wlin ----
            out_ps = ps_mt.tile([1, SLOTS], _f32, name="mt_ps")
            for fo in range(2):
                nc.tensor.matmul(out_ps[:], wl[:, fo:fo + 1], h3_sb[:, fo, :],
                                 start=(fo == 0), stop=(fo == 1))
            out_sb = outp.tile([1, SLOTS], _f32)
            nc.vector.tensor_copy(out_sb[:], out_ps[:])
            nc.sync.dma_start(out_d[:], out_sb[:])

    nc.compile()
    return nc


def _get_nc():
    if "nc" not in _compiled:
        _compiled["nc"] = _build_nc()
    return _compiled["nc"]


def _host_prep(x, edge_weight, W0, W1, Wlin, edge_index):
    bf16 = ml_dtypes.bfloat16
    src = edge_index[0].astype(np.int64)
    tgt = edge_index[1].astype(np.int64)
    b = src // K
    sl = src - b * K
    tl = tgt - (tgt // K) * K

    # dense raw adjacency per graph, indexed [b, t, s]
    idx = (b * K + tl) * K + sl
    Araw = np.bincount(idx, weights=edge_weight.astype(np.float64),
                       minlength=B * K * K).astype(np.float32).reshape(B, K, K)
    deg = Araw.sum(axis=2)                      # weighted in-degree [B, K]
    with np.errstate(divide="ignore"):
        dinv = np.where(deg > 0, 1.0 / np.sqrt(deg), 0.0).astype(np.float32)
    An = Araw * dinv[:, :, None] * dinv[:, None, :]   # [b, t, s]
    ATn = np.ascontiguousarray(An.transpose(0, 2, 1))  # [b, s, t]

    # scatter graphs into per-core padded slots
    ATs = np.zeros((NCORES, SLOTS, K, K), np.float32)
    ATs[:, :GPC] = ATn.reshape(NCORES, GPC, K, K)
    ATs = ATs.reshape(NCORES, NT, G, K, K)

    # block-diagonal AT per tile, zero padded to 128x128
    at = np.zeros((NCORES, NT, TN, TN), np.float32)
    bd = at[:, :, :G * K, :G * K].reshape(NCORES, NT, G, K, G, K)
    atc = np.zeros((NCORES, NT, TN, CP), np.float32)
    cent = atc[:, :, :G * K, :G].reshape(NCORES, NT, G, K, G)
    for g in range(G):
        bd[:, :, g, :, g, :] = ATs[:, :, g]          # [s, t] block
        cent[:, :, g, :, g] = ATs[:, :, g, :, 0]     # center (t_local=0) col
    # partition-major (node-within-tile first) device layout
    at = np.ascontiguousarray(at.transpose(0, 2, 1, 3)).astype(bf16)
    atc = np.ascontiguousarray(atc.transpose(0, 2, 1, 3)).astype(bf16)

    # x node-major per tile: [core, s, tile, f]
    xp = np.zeros((NCORES, NT * G * K, F0), np.float32)
    xp[:, :GPC * K] = x.reshape(NCORES, GPC * K, F0)
    xq = np.zeros((NCORES, NT, TN, F0), np.float32)
    xq[:, :, :G * K] = xp.reshape(NCORES, NT, G * K, F0)
    xq = np.ascontiguousarray(xq.transpose(0, 2, 1, 3)).astype(bf16)

    w1 = np.concatenate([W1[0:128, :], W1[128:256, :]], axis=1).astype(bf16)
    wl = np.ascontiguousarray(Wlin.reshape(2, 128).T).astype(bf16)
    w0 = W0.astype(bf16)

    in_maps = []
    for c in range(NCORES):
        in_maps.append({
            "x": xq[c],
            "at": at[c],
            "atc": atc[c],
            "w0": w0,
            "w1": np.ascontiguousarray(w1),
            "wl": wl,
        })
    return in_maps


def _run(inputs, trace=False):
    nc = _get_nc()
    in_maps = _host_prep(**inputs)
    res = run_bass_kernel_spmd(nc, in_maps, core_ids=list(range(NCORES)),
                               trace=trace)
    out = np.empty((B, 1), np.float32)
    for c in range(NCORES):
        out[c * GPC:(c + 1) * GPC, 0] = res.results[c]["out"][0, :GPC]
    return out, res


def kernel(**inputs):
    out, _ = _run(inputs, trace=False)
    return out


# revision 13
# speedup vs baseline: 1.9549x; 1.0693x over previous
"""Trainium2 Bass kernel for a 2-layer GCN over 2048 independent 25-node
KNN subgraphs (gnn_message_passing).

v3 strategy (v2 measured 45.3us traced; baseline f32r 74us traced):
  - bf16 operands everywhere (FWL weight loads, 1 cy/col matmuls),
    f32 PSUM accumulation. rel err ~8e-3 vs 2e-2 gate.
  - Layer 1 reassociated as (A @ x) @ W0 via mT = X.T-stationary x
    AT-moving (128 cols), then h1 = mT-stationary x W0-moving: 400
    moving cols/tile instead of 528 and half the PSUM->SBUF cast bytes.
  - 4-tile stages: one PSUM tile + one cast + one relu instruction per
    4 tiles (Act/DVE fixed cost is ~130-260ns per instruction).
  - Software-pipelined PE stream: stage s issues mm1(s), mm2(s-1),
    p2(s-2) so the in-order PE never sits behind a cast/relu
    round-trip (v2 lost ~570ns/group to exactly that).
  - Cast and relu alternate between DVE and Act per stage to balance
    ~1650ns of elementwise work across both engines.
  - p2 center columns accumulate 16 tiles per PSUM bank -> 4 copies
    total.
  - DMAs split across the two HWDGE rings (SP: adjacency, Act: x) plus
    the GpSimd SWDGE ring (weights + atc): rings run concurrently, so
    the ~0.9us per-DMA ring turnaround overlaps, and first-needed
    chunks land ~1.5us after the preamble.
  - Short PE warmup bridges the gap until the first chunk arrives so
    the HAM activity monitor unthrottles the PE clock (1.2 -> 2.4 GHz)
    early in the loop instead of 14us in (observed in v2).
  - Data parallel over 8 cores: 256 graphs (52 tiles of 5 graphs) per
    core; weights replicated.
"""

import sys

import ml_dtypes
import numpy as np

for _p in ("/opt/trn_rl_repo", "/opt/trn_rl_repo/concourse"):
    if _p not in sys.path:
        sys.path.insert(0, _p)

import concourse.bass as bass
import concourse.tile as tile
from concourse import bacc, mybir
from concourse.bass_utils import run_bass_kernel_spmd

NCORES = 8
B = 2048            # graphs
K = 25              # nodes per graph
GPC = B // NCORES   # 256 graphs per core
G = 5               # graphs packed per PE tile
NT = (GPC + G - 1) // G   # 52 tiles per core
SLOTS = NT * G      # 260 graph slots per core
TN = 128            # padded nodes per tile (125 real)
CP = 8              # padded center count per tile (5 real)
F0 = 128            # input features
F1 = 256            # hidden features

NTS = 4             # tiles per pipeline stage
NS = NT // NTS      # 13 stages
PB = 16             # tiles per p2 PSUM block
CHUNKS = [4, 8, 8, 16, 16]  # tiles per streamed x/at DMA chunk
WARMUP_TILES = 3    # PE warmup PSUM tiles (4 matmuls each)
D2 = 2              # software-pipeline depth of the W0 stage
D3 = 4              # software-pipeline depth of the p2 stage

_f32 = mybir.dt.float32
_bf16 = mybir.dt.bfloat16

_compiled = {}


def _build_nc():
    nc = bacc.Bacc("TRN2", target_bir_lowering=False, debug=False,
                   num_devices=NCORES)

    # node-major x: partitions = node-within-tile, contiguous per partition
    x_d = nc.dram_tensor("x", [TN, NT, F0], _bf16, kind="ExternalInput")
    # at[s, i, t]: block-diagonal normalized adjacency, zero padded
    at_d = nc.dram_tensor("at", [TN, NT, TN], _bf16, kind="ExternalInput")
    atc_d = nc.dram_tensor("atc", [TN, NT, CP], _bf16, kind="ExternalInput")
    w0_d = nc.dram_tensor("w0", [F0, F1], _bf16, kind="ExternalInput")
    w1_d = nc.dram_tensor("w1", [128, 2 * F1], _bf16, kind="ExternalInput")
    wl_d = nc.dram_tensor("wl", [128, 2], _bf16, kind="ExternalInput")
    out_d = nc.dram_tensor("out", [1, SLOTS], _f32, kind="ExternalOutput")

    relu = mybir.ActivationFunctionType.Relu
    copyf = mybir.ActivationFunctionType.Copy

    with tile.TileContext(nc) as tc:
        with (
            tc.tile_pool(name="const", bufs=1) as cpool,
            tc.tile_pool(name="mtp", bufs=3) as mtp,
            tc.tile_pool(name="h1p", bufs=3) as h1p,
            tc.tile_pool(name="outp", bufs=1) as outp,
            tc.tile_pool(name="ps_mt", bufs=2, space=bass.MemorySpace.PSUM) as ps_mt,
            tc.tile_pool(name="ps_h1", bufs=2, space=bass.MemorySpace.PSUM) as ps_h1,
            tc.tile_pool(name="ps_p2", bufs=2, space=bass.MemorySpace.PSUM) as ps_p2,
        ):
            # ---- resident inputs: three concurrent DMA rings, each FIFO,
            # issued in first-use order ----
            x_all = cpool.tile([TN, NT, F0], _bf16)
            at_all = cpool.tile([TN, NT, TN], _bf16)
            atc_all = cpool.tile([TN, NT, CP], _bf16)
            w0 = cpool.tile([F0, F1], _bf16)
            w1 = cpool.tile([128, 2 * F1], _bf16)
            wl = cpool.tile([128, 2], _bf16)

            nc.gpsimd.dma_start(w0[:], w0_d[:])
            nc.gpsimd.dma_start(atc_all[:], atc_d[:])
            nc.gpsimd.dma_start(w1[:], w1_d[:])
            nc.gpsimd.dma_start(wl[:], wl_d[:])
            bounds = np.cumsum([0] + CHUNKS)
            # all x/at chunks on the SP HWDGE ring, strictly in consumption
            # order: the ring is FIFO so chunk k lands before chunk k+1
            for c in range(len(CHUNKS)):
                lo, hi = bounds[c], bounds[c + 1]
                nc.sync.dma_start(x_all[:, lo:hi, :], x_d[:, lo:hi, :])
                nc.sync.dma_start(at_all[:, lo:hi, :], at_d[:, lo:hi, :])

            # ---- PE warmup on a zeroed tile (no DMA deps): keeps the PE
            # busy until real data lands so HAM unthrottles early ----
            warm = cpool.tile([128, F1], _bf16)
            nc.vector.memset(warm[:], 0)
            for _ in range(WARMUP_TILES):
                wp = ps_h1.tile([128, NTS, F1], _f32, name="h1_ps")
                for j in range(NTS):
                    nc.tensor.matmul(wp[:, j, :], warm[:, 0:128], warm[:],
                                     start=True, stop=True)

            # p2 accumulator: [f-chunk, tile, center], bf16 for the final
            # weight-stationary W1 transform
            p2a = cpool.tile([128, 2, NT, G], _bf16)

            h3_sb = cpool.tile([128, 2, SLOTS], _bf16)

            mt_sbs = {}
            h1_sbs = {}
            p2_ps = None
            # ---- software-pipelined stage loop ----
            for s in range(NS + D3 + 1):
                if s < NS:
                    # mT[f, t] = sum_s x[s, f] * at[s, t]  (= (A @ x).T)
                    mt_ps = ps_mt.tile([128, NTS, TN], _f32)
                    for j in range(NTS):
                        i = s * NTS + j
                        nc.tensor.matmul(mt_ps[:, j, :], x_all[:, i, :],
                                         at_all[:, i, :], start=True, stop=True)
                    mt_sb = mtp.tile([128, NTS, TN], _bf16)
                    if s % 2 == 0:
                        nc.vector.tensor_copy(mt_sb[:], mt_ps[:])
                    else:
                        nc.scalar.activation(mt_sb[:], mt_ps[:], copyf)
                    mt_sbs[s] = mt_sb

                if D2 <= s < NS + D2:
                    # h1[t, fo] = sum_f mT[f, t] * W0[f, fo]
                    g = s - D2
                    mt_sb = mt_sbs.pop(g)
                    h1_ps = ps_h1.tile([128, NTS, F1], _f32, name="h1_ps")
                    for j in range(NTS):
                        nc.tensor.matmul(h1_ps[:, j, :], mt_sb[:, j, :], w0[:],
                                         start=True, stop=True)
                    h1_sb = h1p.tile([128, NTS, F1], _bf16)
                    if s % 2 == 0:
                        nc.scalar.activation(h1_sb[:], h1_ps[:], relu)
                    else:
                        nc.vector.tensor_scalar_max(h1_sb[:], h1_ps[:], 0.0)
                    h1_sbs[g] = h1_sb

                if D3 <= s < NS + D3:
                    # p2T[f, tc] = sum_s h1[s, f] * ATc[s, tc]
                    q = s - D3
                    h1_sb = h1_sbs.pop(q)
                    for j in range(NTS):
                        i = q * NTS + j
                        if i % PB == 0:
                            p2_ps = ps_p2.tile([128, 2, PB, CP], _f32,
                                               name="p2_ps")
                        for c in range(2):
                            nc.tensor.matmul(p2_ps[:, c, i % PB, :],
                                             h1_sb[:, j, c * 128:(c + 1) * 128],
                                             atc_all[:, i, :],
                                             start=True, stop=True)
                        if i % PB == PB - 1 or i == NT - 1:
                            # copy this block's centers out, then run its
                            # W1 transform inline so the tail stays short
                            n = i % PB + 1
                            blk = i // PB
                            o = blk * PB * G
                            w = n * G
                            nc.vector.tensor_copy(
                                p2a[:, :, blk * PB:blk * PB + n, :],
                                p2_ps[:, :, 0:n, 0:G])
                            h3_ps = ps_p2.tile([128, 2, PB * G], _f32,
                                               name="p2_ps")
                            for fo in range(2):
                                for fi in range(2):
                                    nc.tensor.matmul(
                                        h3_ps[:, fo, 0:w],
                                        w1[:, fi * F1 + fo * 128:fi * F1 + fo * 128 + 128],
                                        p2a[:, fi, blk * PB:blk * PB + n, :],
                                        start=(fi == 0), stop=(fi == 1))
                            if blk % 2 == 0:
                                nc.scalar.activation(
                                    h3_sb[:, :, o:o + w], h3_ps[:, :, 0:w],
                                    relu)
                            else:
                                nc.vector.tensor_scalar_max(
                                    h3_sb[:, :, o:o + w], h3_ps[:, :, 0:w],
                                    0.0)

            # ---- out = relu(h3).T @ Wlin ----
            out_ps = ps_mt.tile([1, SLOTS], _f32, name="mt_ps")
            for fo in range(2):
                nc.tensor.matmul(out_ps[:], wl[:, fo:fo + 1], h3_sb[:, fo, :],
                                 start=(fo == 0), stop=(fo == 1))
            out_sb = outp.tile([1, SLOTS], _f32)
            nc.vector.tensor_copy(out_sb[:], out_ps[:])
            nc.sync.dma_start(out_d[:], out_sb[:])

    nc.compile()
    return nc


def _get_nc():
    if "nc" not in _compiled:
        _compiled["nc"] = _build_nc()
    return _compiled["nc"]


def _host_prep(x, edge_weight, W0, W1, Wlin, edge_index):
    bf16 = ml_dtypes.bfloat16
    src = edge_index[0].astype(np.int64)
    tgt = edge_index[1].astype(np.int64)
    b = src // K
    sl = src - b * K
    tl = tgt - (tgt // K) * K

    # dense raw adjacency per graph, indexed [b, t, s]
    idx = (b * K + tl) * K + sl
    Araw = np.bincount(idx, weights=edge_weight.astype(np.float64),
                       minlength=B * K * K).astype(np.float32).reshape(B, K, K)
    deg = Araw.sum(axis=2)                      # weighted in-degree [B, K]
    with np.errstate(divide="ignore"):
        dinv = np.where(deg > 0, 1.0 / np.sqrt(deg), 0.0).astype(np.float32)
    An = Araw * dinv[:, :, None] * dinv[:, None, :]   # [b, t, s]
    ATn = np.ascontiguousarray(An.transpose(0, 2, 1))  # [b, s, t]

    # scatter graphs into per-core padded slots
    ATs = np.zeros((NCORES, SLOTS, K, K), np.float32)
    ATs[:, :GPC] = ATn.reshape(NCORES, GPC, K, K)
    ATs = ATs.reshape(NCORES, NT, G, K, K)

    # block-diagonal AT per tile, zero padded to 128x128
    at = np.zeros((NCORES, NT, TN, TN), np.float32)
    bd = at[:, :, :G * K, :G * K].reshape(NCORES, NT, G, K, G, K)
    atc = np.zeros((NCORES, NT, TN, CP), np.float32)
    cent = atc[:, :, :G * K, :G].reshape(NCORES, NT, G, K, G)
    for g in range(G):
        bd[:, :, g, :, g, :] = ATs[:, :, g]          # [s, t] block
        cent[:, :, g, :, g] = ATs[:, :, g, :, 0]     # center (t_local=0) col
    # partition-major (node-within-tile first) device layout
    at = np.ascontiguousarray(at.transpose(0, 2, 1, 3)).astype(bf16)
    atc = np.ascontiguousarray(atc.transpose(0, 2, 1, 3)).astype(bf16)

    # x node-major per tile: [core, s, tile, f]
    xp = np.zeros((NCORES, NT * G * K, F0), np.float32)
    xp[:, :GPC * K] = x.reshape(NCORES, GPC * K, F0)
    xq = np.zeros((NCORES, NT, TN, F0), np.float32)
    xq[:, :, :G * K] = xp.reshape(NCORES, NT, G * K, F0)
    xq = np.ascontiguousarray(xq.transpose(0, 2, 1, 3)).astype(bf16)

    w1 = np.concatenate([W1[0:128, :], W1[128:256, :]], axis=1).astype(bf16)
    wl = np.ascontiguousarray(Wlin.reshape(2, 128).T).astype(bf16)
    w0 = W0.astype(bf16)

    in_maps = []
    for c in range(NCORES):
        in_maps.append({
            "x": xq[c],
            "at": at[c],
            "atc": atc[c],
            "w0": w0,
            "w1": np.ascontiguousarray(w1),
            "wl": wl,
        })
    return in_maps


def _run(inputs, trace=False):
    nc = _get_nc()
    in_maps = _host_prep(**inputs)
    res = run_bass_kernel_spmd(nc, in_maps, core_ids=list(range(NCORES)),
                               trace=trace)
    out = np.empty((B, 1), np.float32)
    for c in range(NCORES):
        out[c * GPC:(c + 1) * GPC, 0] = res.results[c]["out"][0, :GPC]
    return out, res


def kernel(**inputs):
    out, _ = _run(inputs, trace=False)
    return out
